# revision 2
# baseline (speedup 1.0000x reference)
"""GatedTinyMambaLayer Trainium2 kernel (8-core data-parallel over batch).

The layer is not recurrent: prev_state only enters via tiny per-batch
projections. Per token the whole computation collapses to

    y1 = x @ Wi                       (d_model 32 -> d_state 32)
    t2 = tanh(0.5*y2 + 0.5*bmix_b),   y2 = x @ (Wig + diag(proj_b) Wcg)
    t3 = tanh(0.5*y3 + 0.5*ddg_b),    y3 = x @ Wdg[:32]
    out = x @ (0.5 Wi Wo) + ((y1+bi)*t2) @ (0.5 Wo) + t3 @ (c2 diag(proj_b) Wo)
          + c_out_b
using sigmoid(z) = (1+tanh(z/2))/2 and the exactness-preserving linearization
exp(-0.025 t3 - 0.025) = e^-0.025 (1 - 0.025 t3) + O(3e-4 rel)  (|arg|<=0.025).

Layout: batch row = 8192 tokens split into 4 partition groups of 2048 tokens;
SBUF tile [128, 2048] has partition 32g+f = feature f of token group g (host
pre-transposes).  All matmuls are full 128x128 with block-diagonal weights
(4 identical 32x32 blocks) in float32r (1 cycle/column vs 4 for fp32).
"""
import sys

sys.path.insert(0, "/opt/trn_rl_repo")

import numpy as np

D = 32
B = 64
T = 8192
NCORES = 8
BPC = B // NCORES          # batches per core
G = 4                      # partition groups (token groups per batch row)
CH = T // G                # tokens per group = free dim per batch row
NT = 4                     # supertiles per batch row
NF = CH // NT              # 512 free columns per supertile
BASE_DECAY = 0.05

_CACHE = {}


def _build():
    from concourse import bacc
    import concourse.mybir as mybir
    from concourse.tile import TileContext

    F32 = mybir.dt.float32
    F32R = mybir.dt.float32r
    ACT = mybir.ActivationFunctionType
    ALU = mybir.AluOpType

    nc = bacc.Bacc(None, target_bir_lowering=False)
    X = nc.dram_tensor("x", [BPC, 128, CH], F32R, kind="ExternalInput")
    WC = nc.dram_tensor("wc", [128, 4, 128], F32R, kind="ExternalInput")
    WB = nc.dram_tensor("wb", [128, BPC, 256], F32R, kind="ExternalInput")
    BIAS = nc.dram_tensor("biasv", [128, 2 * BPC + 1], F32, kind="ExternalInput")
    OUT = nc.dram_tensor("out", [BPC, 128, CH], F32, kind="ExternalOutput")

    with TileContext(nc) as tc:
        with (
            tc.tile_pool(name="const", bufs=1) as cpool,
            tc.tile_pool(name="xin", bufs=2) as xpool,
            tc.tile_pool(name="work", bufs=2) as wpool,
            tc.tile_pool(name="outs", bufs=2) as opool,
            tc.tile_pool(name="psum", bufs=2, space="PSUM") as ppool,
        ):
            wc = cpool.tile([128, 4, 128], F32R)
            nc.sync.dma_start(out=wc, in_=WC[:, :, :])
            wb = cpool.tile([128, BPC, 256], F32R)
            nc.sync.dma_start(out=wb, in_=WB[:, :, :])
            bias = cpool.tile([128, 2 * BPC + 1], F32)
            nc.sync.dma_start(out=bias, in_=BIAS[:, :])
            bi_col = bias[:, 2 * BPC:2 * BPC + 1]

            for b in range(BPC):
                xt = xpool.tile([128, CH], F32R, tag="xt")
                nc.sync.dma_start(out=xt, in_=X[b, :, :])
                out_sb = opool.tile([128, CH], F32, tag="osb")
                for s in range(NT):
                    fsl = slice(s * NF, (s + 1) * NF)
                    rhs = xt[:, fsl]
                    y1 = ppool.tile([128, NF], F32, tag="y1")
                    y2 = ppool.tile([128, NF], F32, tag="y2")
                    y3 = ppool.tile([128, NF], F32, tag="y3")
                    nc.tensor.matmul(y1, wc[:, 0, :], rhs, start=True, stop=True)
                    nc.tensor.matmul(y2, wb[:, b, 0:128], rhs, start=True, stop=True)
                    nc.tensor.matmul(y3, wc[:, 1, :], rhs, start=True, stop=True)
                    t2 = wpool.tile([128, NF], F32, tag="t2")
                    nc.scalar.activation(t2, y2, ACT.Tanh,
                                         bias=bias[:, 2 * b:2 * b + 1], scale=0.5)
                    t3 = wpool.tile([128, NF], F32R, tag="t3")
                    nc.scalar.activation(t3, y3, ACT.Tanh,
                                         bias=bias[:, 2 * b + 1:2 * b + 2], scale=0.5)
                    pp = wpool.tile([128, NF], F32R, tag="pp")
                    nc.vector.scalar_tensor_tensor(pp, y1, bi_col, t2,
                                                   op0=ALU.add, op1=ALU.mult)
                    op = ppool.tile([128, NF], F32, tag="op")
                    nc.tensor.matmul(op, wc[:, 2, :], rhs, start=True, stop=False)
                    nc.tensor.matmul(op, wc[:, 3, :], pp, start=False, stop=False)
                    nc.tensor.matmul(op, wb[:, b, 128:256], t3, start=False, stop=True)
                    nc.vector.tensor_copy(out=out_sb[:, fsl], in_=op)
                nc.sync.dma_start(out=OUT[b, :, :], in_=out_sb)
    nc.finalize()
    return nc


def _get_nc():
    if "nc" not in _CACHE:
        _CACHE["nc"] = _build()
    return _CACHE["nc"]


def _bd(w):
    """[32,32] -> [128,128] block-diagonal (4 copies)."""
    out = np.zeros((128, 128), dtype=np.float32)
    for g in range(G):
        out[32 * g:32 * g + 32, 32 * g:32 * g + 32] = w
    return out


def kernel(x, prev_state, Wi, bi, Ws, bs, Wo, bo,
           Wig, big, Wsg, bsg, Wcg, bcg, Wdg, bdg):
    from concourse.bass_utils import run_bass_kernel_spmd

    x = np.asarray(x, dtype=np.float32)
    prev_state = np.asarray(prev_state, dtype=np.float32)
    Wi, bi, Ws, bs = map(np.asarray, (Wi, bi, Ws, bs))
    Wo, bo, Wig, big = map(np.asarray, (Wo, bo, Wig, big))
    Wsg, bsg, Wcg, bcg = map(np.asarray, (Wsg, bsg, Wcg, bcg))
    Wdg, bdg = np.asarray(Wdg), np.asarray(bdg)

    # ---- per-batch host precompute (tiny) ----
    proj = prev_state @ Ws + bs                      # [B, 32]
    sg = proj @ Wsg + bsg                            # [B, 32]
    bmix = big + bcg + sg                            # [B, 32]
    ddg = proj @ Wdg[D:] + bdg                       # [B, 32]
    Wmix = Wig[None] + proj[:, :, None] * Wcg[None]  # [B, 32, 32]
    c1 = np.float32(np.exp(-BASE_DECAY / 2))
    c2 = np.float32(-(BASE_DECAY / 2) * c1)
    WDb = c2 * proj[:, :, None] * Wo[None]           # [B, 32, 32]
    c_out = bo + 0.5 * (bi @ Wo) + c1 * (proj @ Wo)  # [B, 32]

    # ---- device tensors ----
    wconst = np.stack([_bd(Wi), _bd(Wdg[:D]), _bd(0.5 * (Wi @ Wo)), _bd(0.5 * Wo)],
                      axis=1).astype(np.float32)     # [128, 4, 128]
    # x: [B, T, 32] -> per batch [4, 32, 2048] (partition 32g+f, free = token)
    xp = np.ascontiguousarray(
        x.reshape(B, G, CH, D).transpose(0, 1, 3, 2)).reshape(B, 128, CH)

    in_maps = []
    for c in range(NCORES):
        bb = slice(c * BPC, (c + 1) * BPC)
        wb_np = np.zeros((128, BPC, 256), dtype=np.float32)
        bias_np = np.zeros((128, 2 * BPC + 1), dtype=np.float32)
        for j, gb in enumerate(range(c * BPC, (c + 1) * BPC)):
            wb_np[:, j, 0:128] = _bd(Wmix[gb])
            wb_np[:, j, 128:256] = _bd(WDb[gb])
            bias_np[:, 2 * j] = np.tile(0.5 * bmix[gb], G)
            bias_np[:, 2 * j + 1] = np.tile(0.5 * ddg[gb], G)
        bias_np[:, 2 * BPC] = np.tile(bi, G)
        in_maps.append(dict(x=np.ascontiguousarray(xp[bb]), wc=wconst,
                            wb=wb_np, biasv=bias_np))

    nc = _get_nc()
    _CACHE["in_maps"] = in_maps
    res = run_bass_kernel_spmd(nc, in_maps, core_ids=list(range(NCORES)))
    _CACHE["last_res"] = res

    out_dev = np.concatenate([r["out"] for r in res.results], axis=0)  # [B,128,CH]
    out = out_dev.reshape(B, G, D, CH).transpose(0, 1, 3, 2).reshape(B, T, D)
    out = out + c_out[:, None, :]

    # ---- second output: new_state at t = T-1 (exact, host) ----
    xl = x[:, -1, :]                                  # [B, 32]
    ns_l = xl @ Wi + bi
    mix_l = 1.0 / (1.0 + np.exp(-(xl @ Wig + big + sg + (xl * proj) @ Wcg + bcg)))
    d_l = np.concatenate([xl, proj], axis=1) @ Wdg + bdg
    df_l = np.exp(-BASE_DECAY / (1.0 + np.exp(-d_l)))
    last = ns_l * mix_l + proj * df_l

    return out.astype(np.float32), last.astype(np.float32)
